# revision 1
# baseline (speedup 1.0000x reference)
"""Trainium2 Bass kernel for nn_Bert_AvgPooling (segment_reduce + mean + FC).

reference semantics:
    tokens = sequence_output.reshape(B*S, H)              # [32768, 768]
    sums   = segment_sum(tokens, seg_ids, 1537)           # sentinel id 1536
    mean   = sums[:1536] / clause_counts[:, None]
    logits = mean @ fc_w.T + fc_b                         # [1536, 16]

Strategy (8 cores, sharded at clause boundaries => no collective):
  - seg ids are non-decreasing over masked positions, so clauses occupy
    contiguous token ranges.  Core c owns clauses [192c, 192c+192) and
    streams the contiguous token span covering them (sentinel tokens in
    between are harmless: their one-hot rows are all zero).
  - Two persistent PSUM accumulators per core: window A = clauses
    [c0, c0+128), window B = [c0+128, c0+256).  Per 128-token tile a
    one-hot (token x window-clause) is built on DVE with is_equal
    against an iota row; PE matmuls accumulate  psum[c,h] += oh.T @ tok.
    Tiles < NA feed window A, tiles >= NB0 feed window B (bounds are
    compile-time maxima over cores; out-of-window ids match nothing).
  - Windows evacuate through bf16 -> PE transpose -> FC matmul (768->16)
    -> scale by 1/count -> +bias -> plain DMA to this core's 192 output
    rows.  Host just concatenates the 8 shards.  Rows >= 192 of window B
    (neighbor cores' clauses, sentinel) are simply not written out.
"""

import sys

for _p in ("/opt/trn_rl_repo", "/opt/trn_rl_repo/concourse"):
    if _p not in sys.path:
        sys.path.insert(0, _p)

import numpy as np

import concourse.bacc as bacc
import concourse.mybir as mybir
import concourse.tile as tile
from concourse import bass_utils

F32 = mybir.dt.float32
BF16 = mybir.dt.bfloat16

B, S, H, NC = 64, 512, 768, 1536
CORES = 8
OUTR = 256  # output rows per core (window A 128 + window B 128); host slices
PAD_ID = 100000.0

LAST_EXEC_INFO = {}

_PROGRAM_CACHE = {}


def _build(NT2, NA, NB0, loop_iters=0, chunk=4, dma_mode="swdge_split", evac_mode="par"):
    """One program for all cores. NT2 = token tiles per core; window-A
    matmuls for tiles [0, NA); window-B matmuls for tiles [NB0, NT2).

    dma_mode: 'swdge_split' | 'swdge_unsplit' | 'hwdge_act' | 'hwdge_dve'
    | 'hwdge_hybrid'"""
    nc = bacc.Bacc(
        "TRN2",
        target_bir_lowering=False,
        debug=False,
        enable_asserts=False,
        num_devices=CORES,
    )
    tok_d = nc.dram_tensor("tok", [NT2 * 128, H], F32, kind="ExternalInput")
    rel_d = nc.dram_tensor("rel", [128, NT2], F32, kind="ExternalInput")
    fcw_d = nc.dram_tensor("fcw", [128, 6, 16], F32, kind="ExternalInput")
    fcb_d = nc.dram_tensor("fcb", [128, 16], F32, kind="ExternalInput")
    invc_d = nc.dram_tensor("invc", [128, 2], F32, kind="ExternalInput")
    out_d = nc.dram_tensor("out", [OUTR, 16], F32, kind="ExternalOutput")

    from contextlib import ExitStack
    import contextlib

    with tile.TileContext(nc) as tc, ExitStack() as ctx:
        CH = chunk
        hwdge = dma_mode.startswith("hwdge")
        bfp = ctx.enter_context(
            tc.tile_pool(name="tokb", bufs=max(4, (24 if hwdge else 48) // CH))
        )
        cpool = ctx.enter_context(tc.tile_pool(name="const", bufs=1))
        # rel gates the one-hot builds (and thereby the PE): load it FIRST
        rel_s = cpool.tile([128, NT2], F32)
        nc.sync.dma_start(out=rel_s[:], in_=rel_d[:])
        # the first token chunk's SWDGE trigger goes ahead of everything on
        # the gpsimd queue so packets start flowing at the earliest moment
        tb0 = None
        if dma_mode == "swdge_unsplit":
            w0 = min(chunk, NT2)
            tb0 = bfp.tile([128, chunk, H], BF16, tag="tokb")
            nc.gpsimd.dma_start(
                out=tb0[:, :w0, :],
                in_=tok_d[0 : w0 * 128, :].rearrange("(c p) h -> p c h", p=128),
            )
        # iota/identity are generated on-device; no DMA on the critical path
        iota_s = cpool.tile([128, 256], F32)
        nc.gpsimd.iota(
            iota_s[:], [[1, 256]], channel_multiplier=0,
            allow_small_or_imprecise_dtypes=True,
        )
        rowidx = cpool.tile([128, 1], F32)
        nc.gpsimd.iota(
            rowidx[:], [[1, 1]], channel_multiplier=1,
            allow_small_or_imprecise_dtypes=True,
        )
        ident = cpool.tile([128, 128], BF16)
        nc.vector.tensor_scalar(
            out=ident[:], in0=iota_s[:, :128], scalar1=rowidx[:, :1],
            scalar2=None, op0=mybir.AluOpType.is_equal,
        )
        fcw_f = cpool.tile([128, 6, 16], F32)
        nc.sync.dma_start(out=fcw_f[:], in_=fcw_d[:])
        fcw_s = cpool.tile([128, 6, 16], BF16)
        nc.scalar.copy(fcw_s[:], fcw_f[:])
        fcb_s = cpool.tile([128, 16], F32)
        nc.sync.dma_start(out=fcb_s[:], in_=fcb_d[:])
        invc_s = cpool.tile([128, 2], F32)
        nc.sync.dma_start(out=invc_s[:], in_=invc_d[:])

        if hwdge:
            tokfp = ctx.enter_context(tc.tile_pool(name="tokf", bufs=max(3, 16 // CH)))
        ohp = ctx.enter_context(tc.tile_pool(name="oh", bufs=8))
        smallp = ctx.enter_context(tc.tile_pool(name="small", bufs=4))
        evacp = ctx.enter_context(tc.tile_pool(name="evac", bufs=2))
        psW = ctx.enter_context(tc.tile_pool(name="psW", bufs=1, space="PSUM"))
        psT = ctx.enter_context(tc.tile_pool(name="psT", bufs=1, space="PSUM"))
        psF = ctx.enter_context(tc.tile_pool(name="psF", bufs=1, space="PSUM"))

        def evac_serial(ps, wslot):
            sums_bf = evacp.tile([128, H], BF16, tag=f"sums{wslot}")
            pst = psT.tile([128, H], BF16, tag=f"psT{wslot}", space="PSUM")
            sumsT = evacp.tile([128, H], BF16, tag=f"sumsT{wslot}")
            psf = psF.tile([128, 32], F32, tag=f"psF{wslot}", space="PSUM")
            for k in range(6):
                sl = slice(k * 128, (k + 1) * 128)
                nc.scalar.copy(sums_bf[:, sl], ps[:, sl])
                nc.tensor.transpose(pst[:, sl], sums_bf[:, sl], ident[:])
                nc.vector.tensor_copy(sumsT[:, sl], pst[:, sl])
                nc.tensor.matmul(
                    psf[:, :16], sumsT[:, sl], fcw_s[:, k, :],
                    start=(k == 0), stop=(k == 5),
                )
            lg = smallp.tile([128, 16], F32, tag=f"lg{wslot}")
            nc.vector.tensor_scalar(
                out=lg[:], in0=psf[:, :16],
                scalar1=invc_s[:, wslot : wslot + 1], scalar2=None,
                op0=mybir.AluOpType.mult,
            )
            nc.vector.tensor_add(lg[:], lg[:], fcb_s[:])
            nc.sync.dma_start(
                out=out_d[wslot * 128 : (wslot + 1) * 128, :], in_=lg[:]
            )

        def evac_sums(ps, wslot):
            """PSUM window -> FC psum [128,16].  Window A evacuates
            mid-stream: keep its copies OFF the DVE queue (DVE feeds one-hots
            for later chunks; a DVE-queued copy waiting on the A-chain stop
            would stall them and starve the PE).  Window B evacuates at the
            end when DVE is free, so its copies alternate ACT/DVE."""
            use_dve = wslot == 1
            sums_bf = evacp.tile([128, H], BF16, tag=f"sums{wslot}")
            pst = psT.tile([128, H], BF16, tag=f"psT{wslot}", space="PSUM")
            sumsT = evacp.tile([128, H], BF16, tag=f"sumsT{wslot}")
            psf = psF.tile([128, 32], F32, tag=f"psF{wslot}", space="PSUM")
            # bulk PSUM -> bf16 copy (two engine halves when DVE is free),
            # then all transposes, then all FC matmuls: PE never stalls on
            # per-block copy chains
            if use_dve:
                nc.scalar.copy(sums_bf[:, :384], ps[:, :384])
                nc.vector.tensor_copy(sums_bf[:, 384:], ps[:, 384:])
            else:
                nc.scalar.copy(sums_bf[:], ps[:])
            for k in range(6):
                sl = slice(k * 128, (k + 1) * 128)
                nc.tensor.transpose(pst[:, sl], sums_bf[:, sl], ident[:])
            for k in range(6):
                sl = slice(k * 128, (k + 1) * 128)
                if use_dve and k % 2 == 0:
                    nc.vector.tensor_copy(sumsT[:, sl], pst[:, sl])
                else:
                    nc.scalar.copy(sumsT[:, sl], pst[:, sl])
            for k in range(6):
                sl = slice(k * 128, (k + 1) * 128)
                nc.tensor.matmul(
                    psf[:, :16], sumsT[:, sl], fcw_s[:, k, :],
                    start=(k == 0), stop=(k == 5),
                )
            return psf

        def finalize(psf, wslot):
            lg = smallp.tile([128, 16], F32, tag=f"lg{wslot}")
            nc.vector.tensor_scalar(
                out=lg[:], in0=psf[:, :16],
                scalar1=invc_s[:, wslot : wslot + 1], scalar2=None,
                op0=mybir.AluOpType.mult,
            )
            nc.vector.tensor_add(lg[:], lg[:], fcb_s[:])
            nc.sync.dma_start(
                out=out_d[wslot * 128 : (wslot + 1) * 128, :], in_=lg[:]
            )

        def evac_par(ps, wslot):
            finalize(evac_sums(ps, wslot), wslot)

        evac = evac_par if evac_mode == "par" else evac_serial

        loop_cm = tc.For_i(0, loop_iters, 1) if loop_iters else contextlib.nullcontext()
        with loop_cm:
            psA = psW.tile([128, H], F32, tag="psA", space="PSUM")
            psB = psW.tile([128, H], F32, tag="psB", space="PSUM")
            # taper the tail: the last tiles stream as 1-tile chunks so their
            # DMA-completion semaphores fire per tile and the PE drains them
            # as they land, instead of waiting for a whole 4-tile chunk after
            # the final packet
            TAPER = min(5, NT2)
            sizes = []
            rem = NT2 - TAPER
            while rem > 0:
                s = min(CH, rem)
                sizes.append(s)
                rem -= s
            sizes += [1] * TAPER
            offs = [0]
            for s in sizes:
                offs.append(offs[-1] + s)
            for t0, w in zip(offs[:-1], sizes):
                src = tok_d[t0 * 128 : (t0 + w) * 128, :].rearrange(
                    "(c p) h -> p c h", p=128
                )
                ci = t0 // CH
                if t0 == 0 and tb0 is not None:
                    tb = tb0
                elif dma_mode == "swdge_split":
                    tb = bfp.tile([128, CH, H], BF16, tag="tokb")
                    # f32 -> bf16 cast happens inside the DMA engine (SWDGE)
                    nc.gpsimd.dma_start(
                        out=tb[:, :w, : H // 2], in_=src[:, :, : H // 2]
                    )
                    nc.gpsimd.dma_start(
                        out=tb[:, :w, H // 2 :], in_=src[:, :, H // 2 :]
                    )
                elif dma_mode == "swdge_unsplit":
                    tb = bfp.tile([128, CH, H], BF16, tag="tokb")
                    # split by ROWS (tiles), not columns: packet size stays
                    # 1536B, but the first half's completion semaphore fires
                    # ~half a chunk earlier, so the PE starts each burst
                    # sooner and carries less backlog into the stream end
                    h1 = min(2, w)
                    nc.gpsimd.dma_start(out=tb[:, :h1, :], in_=src[:, :h1, :])
                    if w > h1:
                        nc.gpsimd.dma_start(out=tb[:, h1:w, :], in_=src[:, h1:, :])
                else:
                    tb = bfp.tile([128, CH, H], BF16, tag="tokb")
                    tf = tokfp.tile([128, CH, H], F32, tag="tokf")
                    nc.sync.dma_start(out=tf[:, :w, : H // 2], in_=src[:, :, : H // 2])
                    nc.sync.dma_start(out=tf[:, :w, H // 2 :], in_=src[:, :, H // 2 :])
                    if dma_mode == "hwdge_act" or (
                        dma_mode == "hwdge_hybrid" and ci % 2 == 0
                    ):
                        nc.scalar.copy(tb[:, :w, :], tf[:, :w, :])
                    else:
                        nc.vector.tensor_copy(tb[:, :w, :], tf[:, :w, :])
                doA = t0 < NA
                doB = t0 + w > NB0
                if doA:
                    ohA = ohp.tile([128, CH, 128], BF16, tag="ohA")
                    nc.vector.tensor_tensor(
                        out=ohA[:, :w, :],
                        in0=rel_s[:, t0 : t0 + w, None].to_broadcast([128, w, 128]),
                        in1=iota_s[:, None, :128].to_broadcast([128, w, 128]),
                        op=mybir.AluOpType.is_equal,
                    )
                if doB:
                    ohB = ohp.tile([128, CH, 128], BF16, tag="ohB")
                    nc.vector.tensor_tensor(
                        out=ohB[:, :w, :],
                        in0=rel_s[:, t0 : t0 + w, None].to_broadcast([128, w, 128]),
                        in1=iota_s[:, None, 128:].to_broadcast([128, w, 128]),
                        op=mybir.AluOpType.is_equal,
                    )
                for i in range(w):
                    t = t0 + i
                    if t < NA:
                        nc.tensor.matmul(
                            psA[:, :512], ohA[:, i, :], tb[:, i, :512],
                            start=(t == 0), stop=(t == NA - 1),
                        )
                        nc.tensor.matmul(
                            psA[:, 512:], ohA[:, i, :], tb[:, i, 512:],
                            start=(t == 0), stop=(t == NA - 1),
                        )
                    if t >= NB0:
                        nc.tensor.matmul(
                            psB[:, :512], ohB[:, i, :], tb[:, i, :512],
                            start=(t == NB0), stop=(t == NT2 - 1),
                        )
                        nc.tensor.matmul(
                            psB[:, 512:], ohB[:, i, :], tb[:, i, 512:],
                            start=(t == NB0), stop=(t == NT2 - 1),
                        )
                if t0 < NA <= t0 + w:
                    # high priority so the scheduler interleaves window A's
                    # evacuation with the remaining stream instead of
                    # pushing it past the last chunk
                    if evac_mode == "par":
                        with tc.high_priority():
                            psfA = evac_sums(psA, 0)
                    else:
                        evac(psA, 0)
            if evac_mode == "par":
                psfB = evac_sums(psB, 1)
                finalize(psfA, 0)
                finalize(psfB, 1)
            else:
                evac(psB, 1)

    nc.compile()
    return nc


def _prepare(tok, seg, counts, fc_w, fc_b):
    """Host-side metadata: per-core token spans aligned to clause ranges,
    with split clauses chosen to balance span lengths across cores."""
    masked = seg < NC
    ids_m = seg[masked]
    sorted_ok = bool(np.all(np.diff(ids_m) >= 0)) and ids_m.size > 0
    if not sorted_ok:
        # arbitrary seg_ids: materialize tokens grouped by clause id
        order = np.argsort(ids_m, kind="stable")
        pos = np.flatnonzero(masked)[order]
        tok = np.ascontiguousarray(tok[pos])
        seg = ids_m[order]
        masked = np.ones(tok.shape[0], dtype=bool)
    ntok = tok.shape[0]
    mpos = np.flatnonzero(masked)
    idsall = seg[mpos]  # sorted clause id per masked token

    # balanced split clauses: core c covers clauses [splits[c], splits[c+1])
    nm = mpos.size
    splits = [0]
    for c in range(1, CORES):
        tgt = (c * nm) // CORES
        splits.append(int(idsall[min(tgt, nm - 1)]))
    splits.append(NC)
    # ensure strictly increasing (degenerate data)
    for c in range(1, CORES + 1):
        if splits[c] <= splits[c - 1]:
            splits[c] = min(NC, splits[c - 1] + 1)
    cnts = [splits[c + 1] - splits[c] for c in range(CORES)]
    if max(cnts) > OUTR:
        # fall back to uniform clause split
        splits = [c * (NC // CORES) for c in range(CORES)] + [NC]
        cnts = [splits[c + 1] - splits[c] for c in range(CORES)]

    starts, ends = [], []
    for c in range(CORES):
        lo_i = np.searchsorted(idsall, splits[c], side="left")
        hi_i = np.searchsorted(idsall, splits[c + 1], side="left")
        if lo_i == hi_i:
            starts.append(0)
            ends.append(1)
        else:
            starts.append(int(mpos[lo_i]))
            ends.append(int(mpos[hi_i - 1]) + 1)
    spans = [max(1, e - s) for s, e in zip(starts, ends)]
    NT2 = max((sp + 127) // 128 for sp in spans)

    counts_pad = np.ones(NC + 512, dtype=np.float32)
    counts_pad[:NC] = counts
    fcw = np.ascontiguousarray(fc_w.reshape(16, 6, 128).transpose(2, 1, 0))
    fcb = np.broadcast_to(fc_b[None, :], (128, 16)).copy()

    in_maps = []
    NA_max, NB0_min = 1, NT2 - 1
    for c in range(CORES):
        s = starts[c]
        c0 = splits[c]
        need = NT2 * 128
        if s + need <= ntok:
            tok_c = tok[s : s + need]
            rel_flat = seg[s : s + need].astype(np.float32) - c0
        else:
            tok_c = np.zeros((need, H), dtype=np.float32)
            avail = ntok - s
            tok_c[:avail] = tok[s:ntok]
            rel_flat = np.full(need, PAD_ID, dtype=np.float32)
            rel_flat[:avail] = seg[s:ntok].astype(np.float32) - c0
        # out-of-window ids match nothing; keep them far away
        rel_flat = np.where(
            (rel_flat >= 0) & (rel_flat < 256), rel_flat, PAD_ID
        ).astype(np.float32)
        rel = np.ascontiguousarray(rel_flat.reshape(NT2, 128).T)
        inA = (rel >= 0) & (rel < 128)
        inB = (rel >= 128) & (rel < cnts[c])
        tiles_A = np.flatnonzero(inA.any(axis=0))
        tiles_B = np.flatnonzero(inB.any(axis=0))
        if tiles_A.size:
            NA_max = max(NA_max, int(tiles_A[-1]) + 1)
        if tiles_B.size:
            NB0_min = min(NB0_min, int(tiles_B[0]))
        invc = np.ones((128, 2), dtype=np.float32)
        invc[:, 0] = 1.0 / counts_pad[c0 : c0 + 128]
        invc[:, 1] = 1.0 / counts_pad[c0 + 128 : c0 + 256]
        in_maps.append(
            {
                "tok": tok_c if tok_c.flags.c_contiguous else np.ascontiguousarray(tok_c),
                "rel": rel,
                "fcw": fcw,
                "fcb": fcb,
                "invc": invc,
            }
        )
    return in_maps, NT2, NA_max, NB0_min, cnts


def kernel(
    sequence_output,
    fc_w,
    fc_b,
    clause_counts,
    seg_ids,
    n_clauses=NC,
    _loop_iters=0,
    _chunk=4,
    _dma_mode="swdge_unsplit",
    _evac_mode="par",
):
    tok = np.ascontiguousarray(np.asarray(sequence_output, dtype=np.float32)).reshape(
        B * S, H
    )
    fc_w = np.asarray(fc_w, dtype=np.float32)
    fc_b = np.asarray(fc_b, dtype=np.float32)
    counts = np.asarray(clause_counts, dtype=np.float32)
    seg = np.asarray(seg_ids, dtype=np.int32).reshape(-1)

    in_maps, NT2, NA, NB0, cnts = _prepare(tok, seg, counts, fc_w, fc_b)

    key = (NT2, NA, NB0, _loop_iters, _chunk, _dma_mode, _evac_mode)
    nc = _PROGRAM_CACHE.get(key)
    if nc is None:
        nc = _build(
            NT2, NA, NB0, loop_iters=_loop_iters, chunk=_chunk,
            dma_mode=_dma_mode, evac_mode=_evac_mode,
        )
        _PROGRAM_CACHE[key] = nc

    import time

    t0 = time.perf_counter()
    res = bass_utils.run_bass_kernel_spmd(
        nc, in_maps, core_ids=list(range(CORES)), trace=False
    )
    t1 = time.perf_counter()
    LAST_EXEC_INFO.clear()
    LAST_EXEC_INFO.update(
        {
            "wall_s": t1 - t0,
            "NT2": NT2,
            "NA": NA,
            "NB0": NB0,
            "cnts": cnts,
            "nc": nc,
            "in_maps": in_maps,
        }
    )

    shards = [res.results[c]["out"][: cnts[c]] for c in range(CORES)]
    full = np.concatenate(shards, axis=0)[:NC]
    return full.astype(np.float32)



# revision 2
# speedup vs baseline: 1.1990x; 1.1990x over previous
"""Trainium2 Bass kernel for nn_Bert_AvgPooling (segment_reduce + mean + FC).

reference semantics:
    tokens = sequence_output.reshape(B*S, H)              # [32768, 768]
    sums   = segment_sum(tokens, seg_ids, 1537)           # sentinel id 1536
    mean   = sums[:1536] / clause_counts[:, None]
    logits = mean @ fc_w.T + fc_b                         # [1536, 16]

Strategy (8 cores, sharded at clause boundaries => no collective):
  - Only masked tokens matter (~75% of B*S).  Host gathers them densely
    per core (clause-aligned balanced split), staged h-major:
    tok_c[128(h%128), 6(h//128), NTOK(token)] so the DMA streams 2KB
    contiguous lines and the PE can use token slabs as stationary
    weights directly.
  - Reassociation kills the evacuation transpose: logits = ohT(tok W).
    Per 128-token tile: proj[t,16] += slab_s.T @ fcw_s (6 matmuls),
    proj copied PSUM->SBUF (ACT), then logits[c,16] += oh_t.T @ proj
    accumulated across the window's tiles in a tiny PSUM bank.
  - One-hots for both 128-clause windows are built upfront on DVE from
    rel ids (is_equal vs iota), in a few large ops; no per-chunk DVE.
  - Tokens land in ONE persistent SBUF buffer (disjoint chunk regions,
    no pool recycling); SWDGE casts f32->bf16 in flight.
  - Final: scale by 1/count, +bias, DMA out 2x[128,16]; host concats.
"""

import sys

for _p in ("/opt/trn_rl_repo", "/opt/trn_rl_repo/concourse"):
    if _p not in sys.path:
        sys.path.insert(0, _p)

import numpy as np

import concourse.bacc as bacc
import concourse.mybir as mybir
import concourse.tile as tile
from concourse import bass_utils

F32 = mybir.dt.float32
BF16 = mybir.dt.bfloat16
FP16 = mybir.dt.float16

B, S, H, NC = 64, 512, 768, 1536
CORES = 8
OUTR = 256  # output rows per core (window A 128 + window B 128); host slices
PAD_ID = 100000.0

LAST_EXEC_INFO = {}

_PROGRAM_CACHE = {}


def _build(NT, NA, NB0, loop_iters=0, chunk=4, stage="f32", trig=2, dve_blk=8):
    """One program for all cores.

    NT: token tiles per core. Window-A logits matmuls for tiles [0, NA);
    window-B for tiles [NB0, NT).
    stage: 'f32' (SWDGE casts to bf16 in flight) | 'bf16' | 'fp16'
    trig: tiles per DMA trigger (completion granularity within a chunk)
    dve_blk: tiles per one-hot DVE build instruction
    """
    nc = bacc.Bacc(
        "TRN2",
        target_bir_lowering=False,
        debug=False,
        enable_asserts=False,
        num_devices=CORES,
    )
    NTOK = NT * 128
    sdt = {"f32": F32, "bf16": BF16, "fp16": FP16}[stage]
    cdt = BF16 if stage != "fp16" else FP16  # on-chip compute dtype
    tok_d = nc.dram_tensor("tok", [128, 6, NTOK], sdt, kind="ExternalInput")
    rel_d = nc.dram_tensor("rel", [128, NT], F32, kind="ExternalInput")
    fcw_d = nc.dram_tensor("fcw", [128, 6, 16], F32, kind="ExternalInput")
    fcb_d = nc.dram_tensor("fcb", [128, 16], F32, kind="ExternalInput")
    invc_d = nc.dram_tensor("invc", [128, 2], F32, kind="ExternalInput")
    out_d = nc.dram_tensor("out", [OUTR, 16], F32, kind="ExternalOutput")

    from contextlib import ExitStack
    import contextlib

    with tile.TileContext(nc) as tc, ExitStack() as ctx:
        cpool = ctx.enter_context(tc.tile_pool(name="const", bufs=1))
        # token stream: the first trigger goes out before anything else
        tokbuf = cpool.tile([128, 6, NTOK], cdt)
        # chunk boundaries (in tiles)
        sizes = []
        rem = NT
        while rem > 0:
            s = min(chunk, rem)
            sizes.append(s)
            rem -= s
        offs = [0]
        for s in sizes:
            offs.append(offs[-1] + s)
        # per-chunk triggers, split into `trig`-tile pieces for earlier
        # completion semaphores
        trig_spans = []
        for t0, w in zip(offs[:-1], sizes):
            q = t0
            while q < t0 + w:
                e = min(q + trig, t0 + w)
                trig_spans.append((q, e))
                q = e
        for q, e in trig_spans:
            nc.gpsimd.dma_start(
                out=tokbuf[:, :, q * 128 : e * 128],
                in_=tok_d[:, :, q * 128 : e * 128],
            )

        rel_s = cpool.tile([128, NT], F32)
        nc.sync.dma_start(out=rel_s[:], in_=rel_d[:])
        fcw_f = cpool.tile([128, 6, 16], F32)
        nc.sync.dma_start(out=fcw_f[:], in_=fcw_d[:])
        fcb_s = cpool.tile([128, 16], F32)
        nc.sync.dma_start(out=fcb_s[:], in_=fcb_d[:])
        invc_s = cpool.tile([128, 2], F32)
        nc.sync.dma_start(out=invc_s[:], in_=invc_d[:])

        iota_s = cpool.tile([128, 256], F32)
        nc.gpsimd.iota(
            iota_s[:], [[1, 256]], channel_multiplier=0,
            allow_small_or_imprecise_dtypes=True,
        )
        fcw_s = cpool.tile([128, 6, 16], cdt)
        nc.scalar.copy(fcw_s[:], fcw_f[:])

        # one-hot buffers for both windows, built upfront in blocks
        ohA = cpool.tile([128, NA, 128], cdt)
        for b0 in range(0, NA, dve_blk):
            b1 = min(b0 + dve_blk, NA)
            nc.vector.tensor_tensor(
                out=ohA[:, b0:b1, :],
                in0=rel_s[:, b0:b1, None].to_broadcast([128, b1 - b0, 128]),
                in1=iota_s[:, None, :128].to_broadcast([128, b1 - b0, 128]),
                op=mybir.AluOpType.is_equal,
            )
        NB = NT - NB0
        ohB = cpool.tile([128, NB, 128], cdt)
        for b0 in range(0, NB, dve_blk):
            b1 = min(b0 + dve_blk, NB)
            nc.vector.tensor_tensor(
                out=ohB[:, b0:b1, :],
                in0=rel_s[:, NB0 + b0 : NB0 + b1, None].to_broadcast(
                    [128, b1 - b0, 128]
                ),
                in1=iota_s[:, None, 128:].to_broadcast([128, b1 - b0, 128]),
                op=mybir.AluOpType.is_equal,
            )

        proj_sb = cpool.tile([128, NT, 16], cdt)
        psP = ctx.enter_context(tc.tile_pool(name="psP", bufs=2, space="PSUM"))
        psL = ctx.enter_context(tc.tile_pool(name="psL", bufs=1, space="PSUM"))
        smallp = ctx.enter_context(tc.tile_pool(name="small", bufs=1))

        logA = psL.tile([128, 16], F32, tag="logA", space="PSUM")
        logB = psL.tile([128, 16], F32, tag="logB", space="PSUM")

        loop_cm = tc.For_i(0, loop_iters, 1) if loop_iters else contextlib.nullcontext()
        with loop_cm:
            for t0, w in zip(offs[:-1], sizes):
                pp = psP.tile([128, chunk * 16], F32, tag="proj", space="PSUM")
                for i in range(w):
                    t = t0 + i
                    for s6 in range(6):
                        nc.tensor.matmul(
                            pp[:, i * 16 : (i + 1) * 16],
                            tokbuf[:, s6, t * 128 : (t + 1) * 128],
                            fcw_s[:, s6, :],
                            start=(s6 == 0),
                            stop=(s6 == 5),
                        )
                nc.scalar.copy(proj_sb[:, t0 : t0 + w, :], pp[:, : w * 16])
                for i in range(w):
                    t = t0 + i
                    if t < NA:
                        nc.tensor.matmul(
                            logA[:],
                            ohA[:, t, :],
                            proj_sb[:, t, :],
                            start=(t == 0),
                            stop=(t == NA - 1),
                        )
                    if t >= NB0:
                        nc.tensor.matmul(
                            logB[:],
                            ohB[:, t - NB0, :],
                            proj_sb[:, t, :],
                            start=(t == NB0),
                            stop=(t == NT - 1),
                        )
                if t0 < NA <= t0 + w:
                    lgA = smallp.tile([128, 16], F32, tag="lgA")
                    nc.vector.tensor_scalar(
                        out=lgA[:], in0=logA[:],
                        scalar1=invc_s[:, 0:1], scalar2=None,
                        op0=mybir.AluOpType.mult,
                    )
                    nc.vector.tensor_add(lgA[:], lgA[:], fcb_s[:])
                    nc.sync.dma_start(out=out_d[0:128, :], in_=lgA[:])
            lgB = smallp.tile([128, 16], F32, tag="lgB")
            nc.vector.tensor_scalar(
                out=lgB[:], in0=logB[:],
                scalar1=invc_s[:, 1:2], scalar2=None,
                op0=mybir.AluOpType.mult,
            )
            nc.vector.tensor_add(lgB[:], lgB[:], fcb_s[:])
            nc.sync.dma_start(out=out_d[128:256, :], in_=lgB[:])

    nc.compile()
    return nc


def _prepare(tok, seg, counts, fc_w, fc_b, stage="f32"):
    """Host-side: gather masked tokens per core (clause-aligned balanced
    split), stage h-major [128, 6, NTOK]; rel ids, fc weights, counts."""
    masked = seg < NC
    ids_m = seg[masked]
    sorted_ok = bool(np.all(np.diff(ids_m) >= 0)) and ids_m.size > 0
    if not sorted_ok:
        order = np.argsort(ids_m, kind="stable")
        pos = np.flatnonzero(masked)[order]
        tok_m = np.ascontiguousarray(tok[pos])
        ids = ids_m[order]
    else:
        pos = np.flatnonzero(masked)
        tok_m = np.ascontiguousarray(tok[pos])
        ids = ids_m
    nm = ids.size

    # balanced split clauses: core c covers clauses [splits[c], splits[c+1])
    splits = [0]
    for c in range(1, CORES):
        tgt = (c * nm) // CORES
        splits.append(int(ids[min(tgt, nm - 1)]))
    splits.append(NC)
    for c in range(1, CORES + 1):
        if splits[c] <= splits[c - 1]:
            splits[c] = min(NC, splits[c - 1] + 1)
    cnts = [splits[c + 1] - splits[c] for c in range(CORES)]
    if max(cnts) > OUTR:
        splits = [c * (NC // CORES) for c in range(CORES)] + [NC]
        cnts = [splits[c + 1] - splits[c] for c in range(CORES)]

    bounds = np.searchsorted(ids, splits)  # token index ranges per core
    spans = [max(1, bounds[c + 1] - bounds[c]) for c in range(CORES)]
    NT = max((sp + 127) // 128 for sp in spans)
    NTOK = NT * 128

    counts_pad = np.ones(NC + 512, dtype=np.float32)
    counts_pad[:NC] = counts
    fcw = np.ascontiguousarray(fc_w.reshape(16, 6, 128).transpose(2, 1, 0))
    fcb = np.broadcast_to(fc_b[None, :], (128, 16)).copy()

    sdt = {"f32": np.float32, "bf16": None, "fp16": np.float16}[stage]
    if stage == "bf16":
        import ml_dtypes

        sdt = ml_dtypes.bfloat16

    in_maps = []
    NA_max, NB0_min = 1, NT - 1
    for c in range(CORES):
        lo, hi = int(bounds[c]), int(bounds[c + 1])
        n = hi - lo
        c0 = splits[c]
        tk = np.zeros((NTOK, H), dtype=np.float32)
        tk[:n] = tok_m[lo:hi]
        rel_flat = np.full(NTOK, PAD_ID, dtype=np.float32)
        rel_flat[:n] = ids[lo:hi].astype(np.float32) - c0
        rel_flat = np.where(
            (rel_flat >= 0) & (rel_flat < 256), rel_flat, PAD_ID
        ).astype(np.float32)
        rel = np.ascontiguousarray(rel_flat.reshape(NT, 128).T)
        inA = (rel >= 0) & (rel < 128)
        inB = (rel >= 128) & (rel < cnts[c])
        tiles_A = np.flatnonzero(inA.any(axis=0))
        tiles_B = np.flatnonzero(inB.any(axis=0))
        if tiles_A.size:
            NA_max = max(NA_max, int(tiles_A[-1]) + 1)
        if tiles_B.size:
            NB0_min = min(NB0_min, int(tiles_B[0]))
        invc = np.ones((128, 2), dtype=np.float32)
        invc[:, 0] = 1.0 / counts_pad[c0 : c0 + 128]
        invc[:, 1] = 1.0 / counts_pad[c0 + 128 : c0 + 256]
        # h-major staging: [NTOK, 768] -> [128(h%128), 6(h//128), NTOK]
        tok_hm = np.ascontiguousarray(
            tk.reshape(NTOK, 6, 128).transpose(2, 1, 0).astype(sdt)
        )
        in_maps.append(
            {"tok": tok_hm, "rel": rel, "fcw": fcw, "fcb": fcb, "invc": invc}
        )
    return in_maps, NT, NA_max, NB0_min, cnts


def kernel(
    sequence_output,
    fc_w,
    fc_b,
    clause_counts,
    seg_ids,
    n_clauses=NC,
    _loop_iters=0,
    _chunk=4,
    _stage="f32",
    _trig=2,
    _dve_blk=8,
):
    tok = np.ascontiguousarray(np.asarray(sequence_output, dtype=np.float32)).reshape(
        B * S, H
    )
    fc_w = np.asarray(fc_w, dtype=np.float32)
    fc_b = np.asarray(fc_b, dtype=np.float32)
    counts = np.asarray(clause_counts, dtype=np.float32)
    seg = np.asarray(seg_ids, dtype=np.int32).reshape(-1)

    in_maps, NT, NA, NB0, cnts = _prepare(tok, seg, counts, fc_w, fc_b, stage=_stage)

    key = (NT, NA, NB0, _loop_iters, _chunk, _stage, _trig, _dve_blk)
    nc = _PROGRAM_CACHE.get(key)
    if nc is None:
        nc = _build(
            NT, NA, NB0, loop_iters=_loop_iters, chunk=_chunk, stage=_stage,
            trig=_trig, dve_blk=_dve_blk,
        )
        _PROGRAM_CACHE[key] = nc

    import time

    t0 = time.perf_counter()
    res = bass_utils.run_bass_kernel_spmd(
        nc, in_maps, core_ids=list(range(CORES)), trace=False
    )
    t1 = time.perf_counter()
    LAST_EXEC_INFO.clear()
    LAST_EXEC_INFO.update(
        {
            "wall_s": t1 - t0,
            "NT2": NT,
            "NA": NA,
            "NB0": NB0,
            "cnts": cnts,
            "nc": nc,
            "in_maps": in_maps,
        }
    )

    shards = [res.results[c]["out"][: cnts[c]] for c in range(CORES)]
    full = np.concatenate(shards, axis=0)[:NC]
    return full.astype(np.float32)


# revision 3
# speedup vs baseline: 1.7573x; 1.4657x over previous
"""Trainium2 Bass kernel for nn_Bert_AvgPooling (segment_reduce + mean + FC).

reference semantics:
    tokens = sequence_output.reshape(B*S, H)              # [32768, 768]
    sums   = segment_sum(tokens, seg_ids, 1537)           # sentinel id 1536
    mean   = sums[:1536] / clause_counts[:, None]
    logits = mean @ fc_w.T + fc_b                         # [1536, 16]

Strategy (8 cores, sharded at clause boundaries => no collective):
  - Only masked tokens matter (~75% of B*S).  Host gathers them densely
    per core (clause-aligned balanced split) and stages them fp16
    h-major-interleaved: tok[128(h%128), NT, 6(h//128), 128(t)] so each
    DMA trigger covers 3KB-contiguous lines per partition and the PE
    can use token slabs [h, t] as stationary weights directly.
  - Reassociation kills the evacuation transpose: logits = ohT(tok W).
    Per 128-token tile: proj[t,16] += slab_s.T @ fcw_s (6 matmuls),
    proj copied PSUM->SBUF (ACT), then logits[c,16] += oh_t.T @ proj
    accumulated across the window's tiles in a tiny PSUM bank.  The
    logits matmuls of chunk j-1 are emitted after chunk j's proj
    matmuls so the PE never waits on the ACT copy.
  - One-hots for both 128-clause windows are built upfront on DVE from
    rel ids (is_equal vs iota); all consts arrive in ONE packed DMA.
  - Tokens land in ONE persistent SBUF buffer (disjoint chunk regions,
    no pool recycling) via HWDGE triggers alternating sync/scalar HW
    queues (fp16 needs no cast; f32 mode falls back to SWDGE casting).
  - Final: scale by 1/count, +bias, DMA out 2x[128,16]; host concats.
"""

import sys

for _p in ("/opt/trn_rl_repo", "/opt/trn_rl_repo/concourse"):
    if _p not in sys.path:
        sys.path.insert(0, _p)

import numpy as np

import concourse.bacc as bacc
import concourse.mybir as mybir
import concourse.tile as tile
from concourse import bass_utils

F32 = mybir.dt.float32
BF16 = mybir.dt.bfloat16
FP16 = mybir.dt.float16

B, S, H, NC = 64, 512, 768, 1536
CORES = 8
OUTR = 256  # output rows per core (window A 128 + window B 128); host slices
PAD_ID = 100000.0

LAST_EXEC_INFO = {}

_PROGRAM_CACHE = {}


def _build(NT, NA, NB0, loop_iters=0, chunk=4, stage="fp16", trig=2, dve_blk=8,
           queues=2):
    """One program for all cores.

    NT: token tiles per core. Window-A logits matmuls for tiles [0, NA);
    window-B for tiles [NB0, NT).
    stage: 'f32' (SWDGE casts to f16 in flight) | 'bf16' | 'fp16'
    trig: tiles per DMA trigger (completion granularity)
    queues: 1 = sync only, 2 = alternate sync/scalar HWDGE
    """
    nc = bacc.Bacc(
        "TRN2",
        target_bir_lowering=False,
        debug=False,
        enable_asserts=False,
        num_devices=CORES,
    )
    sdt = {"f32": F32, "bf16": BF16, "fp16": FP16}[stage]
    cdt = FP16 if stage != "bf16" else BF16  # on-chip compute dtype
    NCST = NT + 6 * 16 + 16 + 2  # rel | fcw | fcb | invc columns
    tok_d = nc.dram_tensor("tok", [128, NT, 6, 128], sdt, kind="ExternalInput")
    cst_d = nc.dram_tensor("cst", [128, NCST], F32, kind="ExternalInput")
    out_d = nc.dram_tensor("out", [OUTR, 16], F32, kind="ExternalOutput")

    from contextlib import ExitStack
    import contextlib

    with tile.TileContext(nc) as tc, ExitStack() as ctx:
        cpool = ctx.enter_context(tc.tile_pool(name="const", bufs=1))
        tokbuf = cpool.tile([128, NT, 6, 128], cdt)

        # trigger spans (in tiles)
        trig_spans = []
        q = 0
        while q < NT:
            e = min(q + trig, NT)
            trig_spans.append((q, e))
            q = e

        hw_engines = [nc.sync, nc.scalar] if queues == 2 else [nc.sync]
        swdge = stage == "f32"

        def tok_dma(idx, q, e):
            if swdge:
                nc.gpsimd.dma_start(
                    out=tokbuf[:, q:e, :, :], in_=tok_d[:, q:e, :, :]
                )
            else:
                hw_engines[idx % len(hw_engines)].dma_start(
                    out=tokbuf[:, q:e, :, :], in_=tok_d[:, q:e, :, :]
                )

        # first trigger goes out before anything else; consts right after
        tok_dma(0, *trig_spans[0])
        cst_s = cpool.tile([128, NCST], F32)
        nc.sync.dma_start(out=cst_s[:], in_=cst_d[:])
        for i, (q, e) in enumerate(trig_spans[1:], start=1):
            tok_dma(i, q, e)

        rel_s = cst_s[:, 0:NT]
        fcw_f = cst_s[:, NT : NT + 96]
        fcb_s = cst_s[:, NT + 96 : NT + 112]
        invc_s = cst_s[:, NT + 112 : NT + 114]

        iota_s = cpool.tile([128, 256], F32)
        nc.gpsimd.iota(
            iota_s[:], [[1, 256]], channel_multiplier=0,
            allow_small_or_imprecise_dtypes=True,
        )
        fcw_s = cpool.tile([128, 6, 16], cdt)
        nc.vector.tensor_copy(fcw_s[:], fcw_f.rearrange("p (s o) -> p s o", s=6))

        # one-hot buffers for both windows, built upfront in blocks
        ohA = cpool.tile([128, NA, 128], cdt)
        for b0 in range(0, NA, dve_blk):
            b1 = min(b0 + dve_blk, NA)
            nc.vector.tensor_tensor(
                out=ohA[:, b0:b1, :],
                in0=rel_s[:, b0:b1, None].to_broadcast([128, b1 - b0, 128]),
                in1=iota_s[:, None, :128].to_broadcast([128, b1 - b0, 128]),
                op=mybir.AluOpType.is_equal,
            )
        NB = NT - NB0
        ohB = cpool.tile([128, NB, 128], cdt)
        for b0 in range(0, NB, dve_blk):
            b1 = min(b0 + dve_blk, NB)
            nc.vector.tensor_tensor(
                out=ohB[:, b0:b1, :],
                in0=rel_s[:, NB0 + b0 : NB0 + b1, None].to_broadcast(
                    [128, b1 - b0, 128]
                ),
                in1=iota_s[:, None, 128:].to_broadcast([128, b1 - b0, 128]),
                op=mybir.AluOpType.is_equal,
            )

        proj_sb = cpool.tile([128, NT, 16], cdt)
        psP = ctx.enter_context(tc.tile_pool(name="psP", bufs=2, space="PSUM"))
        psL = ctx.enter_context(tc.tile_pool(name="psL", bufs=1, space="PSUM"))
        smallp = ctx.enter_context(tc.tile_pool(name="small", bufs=1))

        logA = psL.tile([128, 16], F32, tag="logA", space="PSUM")
        logB = psL.tile([128, 16], F32, tag="logB", space="PSUM")

        # chunk boundaries (proj-psum granularity)
        sizes = []
        rem = NT
        while rem > 0:
            s = min(chunk, rem)
            sizes.append(s)
            rem -= s
        offs = [0]
        for s in sizes:
            offs.append(offs[-1] + s)

        def emit_logits(t0, w):
            for i in range(w):
                t = t0 + i
                if t < NA:
                    nc.tensor.matmul(
                        logA[:], ohA[:, t, :], proj_sb[:, t, :],
                        start=(t == 0), stop=(t == NA - 1),
                    )
                if t >= NB0:
                    nc.tensor.matmul(
                        logB[:], ohB[:, t - NB0, :], proj_sb[:, t, :],
                        start=(t == NB0), stop=(t == NT - 1),
                    )

        def finalize(log, wslot, lo, hi):
            lg = smallp.tile([128, 16], F32, tag=f"lg{wslot}")
            nc.vector.tensor_scalar(
                out=lg[:], in0=log[:],
                scalar1=invc_s[:, wslot : wslot + 1], scalar2=None,
                op0=mybir.AluOpType.mult,
            )
            nc.vector.tensor_add(lg[:], lg[:], fcb_s[:])
            nc.sync.dma_start(out=out_d[lo:hi, :], in_=lg[:])

        loop_cm = tc.For_i(0, loop_iters, 1) if loop_iters else contextlib.nullcontext()
        with loop_cm:
            prev = None
            for t0, w in zip(offs[:-1], sizes):
                pp = psP.tile([128, chunk * 16], F32, tag="proj", space="PSUM")
                for i in range(w):
                    t = t0 + i
                    for s6 in range(6):
                        nc.tensor.matmul(
                            pp[:, i * 16 : (i + 1) * 16],
                            tokbuf[:, t, s6, :],
                            fcw_s[:, s6, :],
                            start=(s6 == 0),
                            stop=(s6 == 5),
                        )
                nc.scalar.copy(proj_sb[:, t0 : t0 + w, :], pp[:, : w * 16])
                if prev is not None:
                    emit_logits(*prev)
                    if prev[0] < NA <= prev[0] + prev[1]:
                        finalize(logA, 0, 0, 128)
                prev = (t0, w)
            emit_logits(*prev)
            if prev[0] < NA <= prev[0] + prev[1]:
                finalize(logA, 0, 0, 128)
            finalize(logB, 1, 128, 256)

    nc.compile()
    return nc


def _prepare(tok, seg, counts, fc_w, fc_b, stage="fp16"):
    """Host-side: gather masked tokens per core (clause-aligned balanced
    split), stage h-major-interleaved [128, NT, 6, 128]; pack consts."""
    masked = seg < NC
    ids_m = seg[masked]
    sorted_ok = bool(np.all(np.diff(ids_m) >= 0)) and ids_m.size > 0
    if not sorted_ok:
        order = np.argsort(ids_m, kind="stable")
        pos = np.flatnonzero(masked)[order]
        tok_m = np.ascontiguousarray(tok[pos])
        ids = ids_m[order]
    else:
        pos = np.flatnonzero(masked)
        tok_m = np.ascontiguousarray(tok[pos])
        ids = ids_m
    nm = ids.size

    # balanced split clauses: core c covers clauses [splits[c], splits[c+1])
    splits = [0]
    for c in range(1, CORES):
        tgt = (c * nm) // CORES
        splits.append(int(ids[min(tgt, nm - 1)]))
    splits.append(NC)
    for c in range(1, CORES + 1):
        if splits[c] <= splits[c - 1]:
            splits[c] = min(NC, splits[c - 1] + 1)
    cnts = [splits[c + 1] - splits[c] for c in range(CORES)]
    if max(cnts) > OUTR:
        splits = [c * (NC // CORES) for c in range(CORES)] + [NC]
        cnts = [splits[c + 1] - splits[c] for c in range(CORES)]

    bounds = np.searchsorted(ids, splits)  # token index ranges per core
    spans = [max(1, bounds[c + 1] - bounds[c]) for c in range(CORES)]
    NT = max((sp + 127) // 128 for sp in spans)
    NTOK = NT * 128

    counts_pad = np.ones(NC + 512, dtype=np.float32)
    counts_pad[:NC] = counts
    fcw = np.ascontiguousarray(fc_w.reshape(16, 6, 128).transpose(2, 1, 0))
    fcb = np.broadcast_to(fc_b[None, :], (128, 16)).copy()

    if stage == "bf16":
        import ml_dtypes

        sdt = ml_dtypes.bfloat16
    else:
        sdt = {"f32": np.float32, "fp16": np.float16}[stage]

    in_maps = []
    NA_max, NB0_min = 1, NT - 1
    for c in range(CORES):
        lo, hi = int(bounds[c]), int(bounds[c + 1])
        n = hi - lo
        c0 = splits[c]
        tk = np.zeros((NTOK, H), dtype=np.float32)
        tk[:n] = tok_m[lo:hi]
        rel_flat = np.full(NTOK, PAD_ID, dtype=np.float32)
        rel_flat[:n] = ids[lo:hi].astype(np.float32) - c0
        rel_flat = np.where(
            (rel_flat >= 0) & (rel_flat < 256), rel_flat, PAD_ID
        ).astype(np.float32)
        rel = np.ascontiguousarray(rel_flat.reshape(NT, 128).T)
        inA = (rel >= 0) & (rel < 128)
        inB = (rel >= 128) & (rel < cnts[c])
        tiles_A = np.flatnonzero(inA.any(axis=0))
        tiles_B = np.flatnonzero(inB.any(axis=0))
        if tiles_A.size:
            NA_max = max(NA_max, int(tiles_A[-1]) + 1)
        if tiles_B.size:
            NB0_min = min(NB0_min, int(tiles_B[0]))
        invc = np.ones((128, 2), dtype=np.float32)
        invc[:, 0] = 1.0 / counts_pad[c0 : c0 + 128]
        invc[:, 1] = 1.0 / counts_pad[c0 + 128 : c0 + 256]
        # [NTOK, 768] -> [128(h%128), NT, 6(h//128), 128(t)]
        tok_hm = np.ascontiguousarray(
            tk.reshape(NT, 128, 6, 128).transpose(3, 0, 2, 1).astype(sdt)
        )
        cst = np.concatenate(
            [rel, fcw.reshape(128, 96), fcb, invc], axis=1
        ).astype(np.float32)
        in_maps.append({"tok": tok_hm, "cst": np.ascontiguousarray(cst)})
    return in_maps, NT, NA_max, NB0_min, cnts


def kernel(
    sequence_output,
    fc_w,
    fc_b,
    clause_counts,
    seg_ids,
    n_clauses=NC,
    _loop_iters=0,
    _chunk=4,
    _stage="fp16",
    _trig=2,
    _dve_blk=8,
    _queues=2,
):
    tok = np.ascontiguousarray(np.asarray(sequence_output, dtype=np.float32)).reshape(
        B * S, H
    )
    fc_w = np.asarray(fc_w, dtype=np.float32)
    fc_b = np.asarray(fc_b, dtype=np.float32)
    counts = np.asarray(clause_counts, dtype=np.float32)
    seg = np.asarray(seg_ids, dtype=np.int32).reshape(-1)

    in_maps, NT, NA, NB0, cnts = _prepare(tok, seg, counts, fc_w, fc_b, stage=_stage)

    key = (NT, NA, NB0, _loop_iters, _chunk, _stage, _trig, _dve_blk, _queues)
    nc = _PROGRAM_CACHE.get(key)
    if nc is None:
        nc = _build(
            NT, NA, NB0, loop_iters=_loop_iters, chunk=_chunk, stage=_stage,
            trig=_trig, dve_blk=_dve_blk, queues=_queues,
        )
        _PROGRAM_CACHE[key] = nc

    import time

    t0 = time.perf_counter()
    res = bass_utils.run_bass_kernel_spmd(
        nc, in_maps, core_ids=list(range(CORES)), trace=False
    )
    t1 = time.perf_counter()
    LAST_EXEC_INFO.clear()
    LAST_EXEC_INFO.update(
        {
            "wall_s": t1 - t0,
            "NT2": NT,
            "NA": NA,
            "NB0": NB0,
            "cnts": cnts,
            "nc": nc,
            "in_maps": in_maps,
        }
    )

    shards = [res.results[c]["out"][: cnts[c]] for c in range(CORES)]
    full = np.concatenate(shards, axis=0)[:NC]
    return full.astype(np.float32)
